# revision 1
# baseline (speedup 1.0000x reference)
"""CMC InfoNCE score + memory-bank momentum update on 8 Trainium2 cores.

Strategy (SPMD, one program on all 8 cores):
  - The score matrix [256, 4097] is sharded along the K dimension: core i
    computes columns [512*i, 512*i + 513) (adjacent cores overlap by one
    column so every core does identical work; the host keeps 512 columns
    from cores 0-6 and all 513 from core 7).
  - Batch (256) sits on SBUF partitions in two halves of 128, so the
    per-sample normalization scale lines up with per-partition scalar
    operands, and the gathered memory rows [b, k*128:(k+1)*128] multiply
    against an x-row broadcast along the chunk dim at zero data cost.
  - Both memory banks are replicated in every core's HBM; rows are
    fetched with SWDGE indirect DMA, one 128-row gather per score column.
  - exp(dot/(T*(||x||+eps))) is fused into one ScalarE activation with a
    per-partition scale; normalization by the global Z and the scatter of
    the 256 updated bank rows into the 512 MB output copies happen on the
    host during unsharding.
  - The momentum row update (256 rows) is computed on-device; every core
    computes all 256 rows (it is tiny) and the host uses core 0's copy.
"""

import sys

for _p in ("/opt/trn_rl_repo", "/root/.axon_site/_ro/trn_rl_repo"):
    if _p not in sys.path:
        sys.path.insert(0, _p)

import numpy as np

import concourse.bass as bass
import concourse.mybir as mybir
import concourse.tile as tile

F32 = mybir.dt.float32
I32 = mybir.dt.int32

D = 128
B = 256
P = 128
T = 0.5
EPS = 1e-7

N_CORES = 8
NB = 1_000_000
KP1 = 4097
C = 513
CG = 64
STRIDE = 512


def _split_multiwaits(nc, maxw=1):
    """This walrus build encodes at most one sync-wait per instruction;
    move excess waits onto standalone EventSemaphore instructions inserted
    just before, on the same engine (same-stream order preserves timing
    semantics)."""
    n = 0
    for f in nc.m.functions:
        for blk in f.blocks:
            newl = []
            changed = False
            for inst in blk.instructions:
                si = inst.sync_info
                if si is not None and si.on_wait and len(si.on_wait) > maxw:
                    waits = list(si.on_wait)
                    for w in waits[:-maxw]:
                        ev = mybir.InstEventSemaphore(
                            name=f"{inst.name}-wsplit{n}",
                            engine=inst.engine,
                            ins=[],
                            outs=[],
                            sync_info=mybir.SyncInfo(on_wait=[w], on_update=[]),
                        )
                        newl.append(ev)
                        n += 1
                    si.on_wait = waits[-maxw:]
                    changed = True
                newl.append(inst)
            if changed:
                blk.instructions[:] = newl
    return n


def _build_nc():
    nc = bass.Bass()
    mem_l = nc.declare_dram_parameter("memory_l", [NB, D], F32, isOutput=False)
    mem_ab = nc.declare_dram_parameter("memory_ab", [NB, D], F32, isOutput=False)
    l_in = nc.declare_dram_parameter("l", [B, D], F32, isOutput=False)
    ab_in = nc.declare_dram_parameter("ab", [B, D], F32, isOutput=False)
    idx_in = nc.declare_dram_parameter("idx_slice", [B, C], I32, isOutput=False)
    y_in = nc.declare_dram_parameter("y", [B, 1], I32, isOutput=False)

    out_l = nc.declare_dram_parameter("out_l", [B, C], F32, isOutput=True)
    out_ab = nc.declare_dram_parameter("out_ab", [B, C], F32, isOutput=True)
    upd_l = nc.declare_dram_parameter("upd_l", [B, D], F32, isOutput=True)
    upd_ab = nc.declare_dram_parameter("upd_ab", [B, D], F32, isOutput=True)

    chunks = []
    c0 = 0
    while c0 < C:
        chunks.append((c0, min(CG, C - c0)))
        c0 += CG

    import dataclasses

    def bcast_chunk(ap, cg):
        # [128, D] -> [128, cg, D] with middle step 0 (free-dim broadcast)
        return dataclasses.replace(ap, ap=[ap.ap[0], [0, cg], ap.ap[1]])

    with tile.TileContext(nc) as tc:
        with (
            tc.tile_pool(name="persist", bufs=1) as pp,
            tc.tile_pool(name="gather", bufs=3) as gp,
            tc.tile_pool(name="scratch", bufs=2) as sp,
            tc.tile_pool(name="small", bufs=2) as mp,
        ):
            xh, rln, rlnT = {}, {}, {}
            for mod, src in (("l", l_in), ("ab", ab_in)):
                for h in (0, 1):
                    xt = pp.tile([P, D], F32, tag=f"x_{mod}_{h}")
                    nc.sync.dma_start(out=xt[:], in_=src[h * P:(h + 1) * P, :])
                    ss = pp.tile([P, 1], F32, tag=f"ss_{mod}_{h}")
                    scr = sp.tile([P, D], F32, tag="normscr")
                    nc.scalar.activation(
                        out=scr[:], in_=xt[:],
                        func=mybir.ActivationFunctionType.Square,
                        accum_out=ss[:],
                    )
                    nrm = pp.tile([P, 1], F32, tag=f"nrm_{mod}_{h}")
                    nc.scalar.sqrt(out=nrm[:], in_=ss[:])
                    nc.vector.tensor_scalar_add(out=nrm[:], in0=nrm[:], scalar1=EPS)
                    rt = pp.tile([P, 1], F32, tag=f"rln_{mod}_{h}")
                    nc.vector.reciprocal(out=rt[:], in_=nrm[:])
                    rtT = pp.tile([P, 1], F32, tag=f"rlnT_{mod}_{h}")
                    nc.vector.tensor_scalar_mul(out=rtT[:], in0=rt[:], scalar1=1.0 / T)
                    xh[(mod, h)] = xt
                    rln[(mod, h)] = rt
                    rlnT[(mod, h)] = rtT

            # scores: out_l pairs memory_ab[idx] with l; out_ab memory_l with ab
            for mod, bank in (("l", mem_ab), ("ab", mem_l)):
                for h in (0, 1):
                    e_t = pp.tile([P, C], F32, tag=f"E_{mod}_{h}")
                    for (c0, cg) in chunks:
                        it = mp.tile([P, CG], I32, tag="idx")
                        nc.sync.dma_start(
                            out=it[:, :cg],
                            in_=idx_in[h * P:(h + 1) * P, c0:c0 + cg],
                        )
                        g = gp.tile([P, CG * D], F32, tag="g")
                        # HW indirect DMA honors one offset per partition:
                        # one gather per score column into slices of one tile.
                        for j in range(cg):
                            nc.gpsimd.indirect_dma_start(
                                out=g[:, j * D:(j + 1) * D],
                                out_offset=None,
                                in_=bank[:],
                                in_offset=bass.IndirectOffsetOnAxis(
                                    ap=it[:, j:j + 1], axis=0,
                                ),
                            )
                        g3 = g[:, :cg * D].rearrange("p (c d) -> p c d", d=D)
                        m = sp.tile([P, CG * D], F32, tag="m")
                        nc.vector.tensor_tensor(
                            out=m[:, :cg * D],
                            in0=g3,
                            in1=bcast_chunk(xh[(mod, h)][:], cg),
                            op=mybir.AluOpType.mult,
                        )
                        s = mp.tile([P, CG], F32, tag="s")
                        nc.vector.tensor_reduce(
                            out=s[:, :cg],
                            in_=m[:, :cg * D].rearrange("p (c d) -> p c d", d=D),
                            axis=mybir.AxisListType.X,
                            op=mybir.AluOpType.add,
                        )
                        nc.scalar.activation(
                            out=e_t[:, c0:c0 + cg],
                            in_=s[:, :cg],
                            func=mybir.ActivationFunctionType.Exp,
                            scale=rlnT[(mod, h)][:],
                        )
                    out_dram = out_l if mod == "l" else out_ab
                    nc.sync.dma_start(out=out_dram[h * P:(h + 1) * P, :], in_=e_t[:])

            # momentum updates: every core computes all 256 rows
            for mod, bank, outd in (("l", mem_l, upd_l), ("ab", mem_ab, upd_ab)):
                for h in (0, 1):
                    yt = mp.tile([P, 1], I32, tag="y")
                    nc.sync.dma_start(out=yt[:], in_=y_in[h * P:(h + 1) * P, :])
                    gy = sp.tile([P, D], F32, tag="gy")
                    nc.gpsimd.indirect_dma_start(
                        out=gy[:], out_offset=None, in_=bank[:],
                        in_offset=bass.IndirectOffsetOnAxis(ap=yt[:, :1], axis=0),
                    )
                    xn = sp.tile([P, D], F32, tag="xn")
                    nc.vector.tensor_scalar_mul(
                        out=xn[:], in0=xh[(mod, h)][:], scalar1=rln[(mod, h)][:]
                    )
                    pos = sp.tile([P, D], F32, tag="pos")
                    nc.vector.tensor_tensor(
                        out=pos[:], in0=gy[:], in1=xn[:], op=mybir.AluOpType.add
                    )
                    nc.vector.tensor_scalar_mul(out=pos[:], in0=pos[:], scalar1=0.5)
                    scr2 = sp.tile([P, D], F32, tag="scr2")
                    ssu = mp.tile([P, 1], F32, tag="ssu")
                    nc.scalar.activation(
                        out=scr2[:], in_=pos[:],
                        func=mybir.ActivationFunctionType.Square,
                        accum_out=ssu[:],
                    )
                    nrmu = mp.tile([P, 1], F32, tag="nrmu")
                    nc.scalar.sqrt(out=nrmu[:], in_=ssu[:])
                    rnu = mp.tile([P, 1], F32, tag="rnu")
                    nc.vector.reciprocal(out=rnu[:], in_=nrmu[:])
                    nc.vector.tensor_scalar_mul(out=pos[:], in0=pos[:], scalar1=rnu[:])
                    nc.sync.dma_start(out=outd[h * P:(h + 1) * P, :], in_=pos[:])

    _split_multiwaits(nc)
    return nc


_cache = {}


def kernel(l, ab, memory_l, memory_ab, y, idx):
    from concourse import bass_utils

    if "nc" not in _cache:
        _cache["nc"] = _build_nc()
    nc = _cache["nc"]

    l = np.ascontiguousarray(np.asarray(l, dtype=np.float32))
    ab = np.ascontiguousarray(np.asarray(ab, dtype=np.float32))
    memory_l = np.ascontiguousarray(np.asarray(memory_l, dtype=np.float32))
    memory_ab = np.ascontiguousarray(np.asarray(memory_ab, dtype=np.float32))
    y = np.asarray(y, dtype=np.int32)
    idx = np.asarray(idx, dtype=np.int32)

    in_maps = []
    for i in range(N_CORES):
        k0 = i * STRIDE
        in_maps.append({
            "memory_l": memory_l,
            "memory_ab": memory_ab,
            "l": l,
            "ab": ab,
            "idx_slice": np.ascontiguousarray(idx[:, k0:k0 + C]),
            "y": y[:, None],
        })

    res = bass_utils.run_bass_kernel_spmd(
        nc, in_maps, core_ids=list(range(N_CORES))
    )
    results = res.results

    def stitch(name):
        full = np.empty((B, KP1), np.float32)
        for i in range(N_CORES):
            full[:, i * STRIDE:i * STRIDE + STRIDE] = results[i][name][:, :STRIDE]
        full[:, KP1 - 1] = results[N_CORES - 1][name][:, STRIDE]
        return full

    e_l = stitch("out_l")
    e_ab = stitch("out_ab")
    z_l = np.float32(e_l.mean(dtype=np.float64) * NB)
    z_ab = np.float32(e_ab.mean(dtype=np.float64) * NB)
    out_l = (e_l / z_l)[..., None].astype(np.float32)
    out_ab = (e_ab / z_ab)[..., None].astype(np.float32)

    new_l = memory_l.copy()
    new_l[y] = results[0]["upd_l"]
    new_ab = memory_ab.copy()
    new_ab[y] = results[0]["upd_ab"]
    return out_l, out_ab, new_l, new_ab


# revision 3
# speedup vs baseline: 1.0006x; 1.0006x over previous
"""CMC InfoNCE score + memory-bank momentum update on 8 Trainium2 cores.

Strategy (SPMD, one program on all 8 cores):
  - The score matrix [256, 4097] is sharded along the K dimension: core i
    computes columns [512*i, 512*i + 513) (adjacent cores overlap by one
    column so every core does identical work; the host keeps 512 columns
    from cores 0-6 and all 513 from core 7).
  - Batch (256) sits on SBUF partitions in two halves of 128, so the
    per-sample normalization scale lines up with per-partition scalar
    operands, and the gathered memory rows [b, k*128:(k+1)*128] multiply
    against an x-row broadcast along the chunk dim at zero data cost.
  - Both memory banks are replicated in every core's HBM; rows are
    fetched with SWDGE indirect DMA, one 128-row gather per score column.
  - exp(dot/(T*(||x||+eps))) is fused into one ScalarE activation with a
    per-partition scale; normalization by the global Z and the scatter of
    the 256 updated bank rows into the 512 MB output copies happen on the
    host during unsharding.
  - The momentum row update (256 rows) is computed on-device; every core
    computes all 256 rows (it is tiny) and the host uses core 0's copy.
"""

import sys

for _p in ("/opt/trn_rl_repo", "/root/.axon_site/_ro/trn_rl_repo"):
    if _p not in sys.path:
        sys.path.insert(0, _p)

import numpy as np

import concourse.bass as bass
import concourse.mybir as mybir
import concourse.tile as tile

F32 = mybir.dt.float32
I32 = mybir.dt.int32

D = 128
B = 256
P = 128
T = 0.5
EPS = 1e-7

N_CORES = 8
NB = 1_000_000
KP1 = 4097
C = 513
CG = 64
STRIDE = 512


def _split_multiwaits(nc, maxw=1):
    """This walrus build encodes at most one sync-wait per instruction;
    move excess waits onto standalone EventSemaphore instructions inserted
    just before, on the same engine (same-stream order preserves timing
    semantics)."""
    n = 0
    for f in nc.m.functions:
        for blk in f.blocks:
            newl = []
            changed = False
            for inst in blk.instructions:
                si = inst.sync_info
                if si is not None and si.on_wait and len(si.on_wait) > maxw:
                    waits = list(si.on_wait)
                    for w in waits[:-maxw]:
                        ev = mybir.InstEventSemaphore(
                            name=f"{inst.name}-wsplit{n}",
                            engine=inst.engine,
                            ins=[],
                            outs=[],
                            sync_info=mybir.SyncInfo(on_wait=[w], on_update=[]),
                        )
                        newl.append(ev)
                        n += 1
                    si.on_wait = waits[-maxw:]
                    changed = True
                newl.append(inst)
            if changed:
                blk.instructions[:] = newl
    return n


def _build_nc():
    nc = bass.Bass()
    mem_l = nc.declare_dram_parameter("memory_l", [NB, D], F32, isOutput=False)
    mem_ab = nc.declare_dram_parameter("memory_ab", [NB, D], F32, isOutput=False)
    l_in = nc.declare_dram_parameter("l", [B, D], F32, isOutput=False)
    ab_in = nc.declare_dram_parameter("ab", [B, D], F32, isOutput=False)
    idx_in = nc.declare_dram_parameter("idx_slice", [B, C], I32, isOutput=False)
    y_in = nc.declare_dram_parameter("y", [B, 1], I32, isOutput=False)

    out_l = nc.declare_dram_parameter("out_l", [B, C], F32, isOutput=True)
    out_ab = nc.declare_dram_parameter("out_ab", [B, C], F32, isOutput=True)
    upd_l = nc.declare_dram_parameter("upd_l", [B, D], F32, isOutput=True)
    upd_ab = nc.declare_dram_parameter("upd_ab", [B, D], F32, isOutput=True)

    chunks = []
    c0 = 0
    while c0 < C:
        chunks.append((c0, min(CG, C - c0)))
        c0 += CG

    import dataclasses

    def bcast_chunk(ap, cg):
        # [128, D] -> [128, cg, D] with middle step 0 (free-dim broadcast)
        return dataclasses.replace(ap, ap=[ap.ap[0], [0, cg], ap.ap[1]])

    with tile.TileContext(nc) as tc:
        with (
            tc.tile_pool(name="persist", bufs=1) as pp,
            tc.tile_pool(name="gather", bufs=3) as gp,
            tc.tile_pool(name="scratch", bufs=2) as sp,
            tc.tile_pool(name="small", bufs=2) as mp,
        ):
            xh, rln, rlnT = {}, {}, {}
            for mod, src in (("l", l_in), ("ab", ab_in)):
                for h in (0, 1):
                    xt = pp.tile([P, D], F32, tag=f"x_{mod}_{h}")
                    nc.sync.dma_start(out=xt[:], in_=src[h * P:(h + 1) * P, :])
                    ss = pp.tile([P, 1], F32, tag=f"ss_{mod}_{h}")
                    scr = sp.tile([P, D], F32, tag="normscr")
                    nc.scalar.activation(
                        out=scr[:], in_=xt[:],
                        func=mybir.ActivationFunctionType.Square,
                        accum_out=ss[:],
                    )
                    nrm = pp.tile([P, 1], F32, tag=f"nrm_{mod}_{h}")
                    nc.scalar.sqrt(out=nrm[:], in_=ss[:])
                    nc.vector.tensor_scalar_add(out=nrm[:], in0=nrm[:], scalar1=EPS)
                    rt = pp.tile([P, 1], F32, tag=f"rln_{mod}_{h}")
                    nc.vector.reciprocal(out=rt[:], in_=nrm[:])
                    rtT = pp.tile([P, 1], F32, tag=f"rlnT_{mod}_{h}")
                    nc.vector.tensor_scalar_mul(out=rtT[:], in0=rt[:], scalar1=1.0 / T)
                    xh[(mod, h)] = xt
                    rln[(mod, h)] = rt
                    rlnT[(mod, h)] = rtT

            # scores: out_l pairs memory_ab[idx] with l; out_ab memory_l with ab
            for mod, bank in (("l", mem_ab), ("ab", mem_l)):
                for h in (0, 1):
                    e_t = pp.tile([P, C], F32, tag=f"E_{mod}_{h}")
                    for (c0, cg) in chunks:
                        it = mp.tile([P, CG], I32, tag="idx")
                        nc.sync.dma_start(
                            out=it[:, :cg],
                            in_=idx_in[h * P:(h + 1) * P, c0:c0 + cg],
                        )
                        g = gp.tile([P, CG * D], F32, tag="g")
                        # HW indirect DMA honors one offset per partition:
                        # one gather per score column into slices of one tile.
                        for j in range(cg):
                            nc.gpsimd.indirect_dma_start(
                                out=g[:, j * D:(j + 1) * D],
                                out_offset=None,
                                in_=bank[:],
                                in_offset=bass.IndirectOffsetOnAxis(
                                    ap=it[:, j:j + 1], axis=0,
                                ),
                            )
                        g3 = g[:, :cg * D].rearrange("p (c d) -> p c d", d=D)
                        m = sp.tile([P, CG * D], F32, tag="m")
                        nc.vector.tensor_tensor(
                            out=m[:, :cg * D],
                            in0=g3,
                            in1=bcast_chunk(xh[(mod, h)][:], cg),
                            op=mybir.AluOpType.mult,
                        )
                        s = mp.tile([P, CG], F32, tag="s")
                        nc.vector.tensor_reduce(
                            out=s[:, :cg],
                            in_=m[:, :cg * D].rearrange("p (c d) -> p c d", d=D),
                            axis=mybir.AxisListType.X,
                            op=mybir.AluOpType.add,
                        )
                        nc.scalar.activation(
                            out=e_t[:, c0:c0 + cg],
                            in_=s[:, :cg],
                            func=mybir.ActivationFunctionType.Exp,
                            scale=rlnT[(mod, h)][:],
                        )
                    out_dram = out_l if mod == "l" else out_ab
                    nc.sync.dma_start(out=out_dram[h * P:(h + 1) * P, :], in_=e_t[:])

            # momentum updates: every core computes all 256 rows
            for mod, bank, outd in (("l", mem_l, upd_l), ("ab", mem_ab, upd_ab)):
                for h in (0, 1):
                    yt = mp.tile([P, 1], I32, tag="y")
                    nc.sync.dma_start(out=yt[:], in_=y_in[h * P:(h + 1) * P, :])
                    gy = sp.tile([P, D], F32, tag="gy")
                    nc.gpsimd.indirect_dma_start(
                        out=gy[:], out_offset=None, in_=bank[:],
                        in_offset=bass.IndirectOffsetOnAxis(ap=yt[:, :1], axis=0),
                    )
                    xn = sp.tile([P, D], F32, tag="xn")
                    nc.vector.tensor_scalar_mul(
                        out=xn[:], in0=xh[(mod, h)][:], scalar1=rln[(mod, h)][:]
                    )
                    pos = sp.tile([P, D], F32, tag="pos")
                    nc.vector.tensor_tensor(
                        out=pos[:], in0=gy[:], in1=xn[:], op=mybir.AluOpType.add
                    )
                    nc.vector.tensor_scalar_mul(out=pos[:], in0=pos[:], scalar1=0.5)
                    scr2 = sp.tile([P, D], F32, tag="scr2")
                    ssu = mp.tile([P, 1], F32, tag="ssu")
                    nc.scalar.activation(
                        out=scr2[:], in_=pos[:],
                        func=mybir.ActivationFunctionType.Square,
                        accum_out=ssu[:],
                    )
                    nrmu = mp.tile([P, 1], F32, tag="nrmu")
                    nc.scalar.sqrt(out=nrmu[:], in_=ssu[:])
                    rnu = mp.tile([P, 1], F32, tag="rnu")
                    nc.vector.reciprocal(out=rnu[:], in_=nrmu[:])
                    nc.vector.tensor_scalar_mul(out=pos[:], in0=pos[:], scalar1=rnu[:])
                    nc.sync.dma_start(out=outd[h * P:(h + 1) * P, :], in_=pos[:])

    _split_multiwaits(nc)
    return nc


_cache = {}


def kernel(l, ab, memory_l, memory_ab, y, idx, _trace=False):
    from concourse import bass_utils

    if "nc" not in _cache:
        _cache["nc"] = _build_nc()
    nc = _cache["nc"]

    l = np.ascontiguousarray(np.asarray(l, dtype=np.float32))
    ab = np.ascontiguousarray(np.asarray(ab, dtype=np.float32))
    memory_l = np.ascontiguousarray(np.asarray(memory_l, dtype=np.float32))
    memory_ab = np.ascontiguousarray(np.asarray(memory_ab, dtype=np.float32))
    y = np.asarray(y, dtype=np.int32)
    idx = np.asarray(idx, dtype=np.int32)

    in_maps = []
    for i in range(N_CORES):
        k0 = i * STRIDE
        in_maps.append({
            "memory_l": memory_l,
            "memory_ab": memory_ab,
            "l": l,
            "ab": ab,
            "idx_slice": np.ascontiguousarray(idx[:, k0:k0 + C]),
            "y": y[:, None],
        })

    res = bass_utils.run_bass_kernel_spmd(
        nc, in_maps, core_ids=list(range(N_CORES)), trace=_trace
    )
    results = res.results
    _cache["last_exec_time_ns"] = res.exec_time_ns

    def stitch(name):
        full = np.empty((B, KP1), np.float32)
        for i in range(N_CORES):
            full[:, i * STRIDE:i * STRIDE + STRIDE] = results[i][name][:, :STRIDE]
        full[:, KP1 - 1] = results[N_CORES - 1][name][:, STRIDE]
        return full

    e_l = stitch("out_l")
    e_ab = stitch("out_ab")
    z_l = np.float32(e_l.mean(dtype=np.float64) * NB)
    z_ab = np.float32(e_ab.mean(dtype=np.float64) * NB)
    out_l = (e_l / z_l)[..., None].astype(np.float32)
    out_ab = (e_ab / z_ab)[..., None].astype(np.float32)

    new_l = memory_l.copy()
    new_l[y] = results[0]["upd_l"]
    new_ab = memory_ab.copy()
    new_ab[y] = results[0]["upd_ab"]
    return out_l, out_ab, new_l, new_ab


# revision 5
# speedup vs baseline: 1.8263x; 1.8252x over previous
"""CMC InfoNCE score + memory-bank momentum update on 8 Trainium2 cores.

Strategy (SPMD, one program on all 8 cores):
  - The score matrix [256, 4097] is sharded along the K dimension: core i
    computes columns [512*i, 512*i + 513) (adjacent cores overlap by one
    column so every core does identical work; the host keeps 512 columns
    from cores 0-6 and all 513 from core 7).
  - Batch (256) sits on SBUF partitions in two halves of 128, so the
    per-sample normalization scale lines up with per-partition scalar
    operands, and the gathered memory rows [b, k*128:(k+1)*128] multiply
    against an x-row broadcast along the chunk dim at zero data cost.
  - Both banks share the same gather indices, so the host interleaves
    them into one [1M, 256] tensor; each indirect-DMA offset then fetches
    1024 B covering both banks' row, halving the instruction count of the
    per-ref gather (which is bound at ~1.4 us/instruction by SWDGE).
  - The per-sample scale 1/(T*(||x||+eps)) is folded into the multiply
    operand, dots for both modalities come out of one segmented reduce,
    and exp is one ScalarE activation per modality over a strided view.
  - Normalization by the global Z and the scatter of the 256 updated rows
    into the 512 MB output copies happen on the host during unsharding.
  - The momentum row update (256 rows) is computed on-device; every core
    computes all 256 rows (it is tiny) and the host uses core 0's copy.
"""

import sys

for _p in ("/opt/trn_rl_repo", "/root/.axon_site/_ro/trn_rl_repo"):
    if _p not in sys.path:
        sys.path.insert(0, _p)

import dataclasses

import numpy as np

import concourse.bass as bass
import concourse.mybir as mybir
import concourse.tile as tile

F32 = mybir.dt.float32
I32 = mybir.dt.int32

D = 128
B = 256
P = 128
T = 0.5
EPS = 1e-7

N_CORES = 8
NB = 1_000_000
KP1 = 4097
C = 513
CG = 32
STRIDE = 512


def _split_multiwaits(nc, maxw=1):
    """This walrus build encodes at most one sync-wait per instruction;
    move excess waits onto standalone EventSemaphore instructions inserted
    just before, on the same engine (same-stream order preserves timing
    semantics)."""
    n = 0
    for f in nc.m.functions:
        for blk in f.blocks:
            newl = []
            changed = False
            for inst in blk.instructions:
                si = inst.sync_info
                if si is not None and si.on_wait and len(si.on_wait) > maxw:
                    waits = list(si.on_wait)
                    for w in waits[:-maxw]:
                        ev = mybir.InstEventSemaphore(
                            name=f"{inst.name}-wsplit{n}",
                            engine=inst.engine,
                            ins=[],
                            outs=[],
                            sync_info=mybir.SyncInfo(on_wait=[w], on_update=[]),
                        )
                        newl.append(ev)
                        n += 1
                    si.on_wait = waits[-maxw:]
                    changed = True
                newl.append(inst)
            if changed:
                blk.instructions[:] = newl
    return n


def _build_nc(split=True):
    D2 = 2 * D  # interleaved row: [memory_ab | memory_l]
    nc = bass.Bass()
    mem_cat = nc.declare_dram_parameter("mem_cat", [NB, D2], F32, isOutput=False)
    l_in = nc.declare_dram_parameter("l", [B, D], F32, isOutput=False)
    ab_in = nc.declare_dram_parameter("ab", [B, D], F32, isOutput=False)
    idx_in = nc.declare_dram_parameter("idx_slice", [B, C], I32, isOutput=False)
    y_in = nc.declare_dram_parameter("y", [B, 1], I32, isOutput=False)

    out_l = nc.declare_dram_parameter("out_l", [B, C], F32, isOutput=True)
    out_ab = nc.declare_dram_parameter("out_ab", [B, C], F32, isOutput=True)
    upd_l = nc.declare_dram_parameter("upd_l", [B, D], F32, isOutput=True)
    upd_ab = nc.declare_dram_parameter("upd_ab", [B, D], F32, isOutput=True)

    chunks = []
    c0 = 0
    while c0 < C:
        chunks.append((c0, min(CG, C - c0)))
        c0 += CG

    def bcast_chunk(ap, cg):
        # [128, D2] -> [128, cg, D2] with middle step 0 (free-dim broadcast)
        return dataclasses.replace(ap, ap=[ap.ap[0], [0, cg], ap.ap[1]])

    with tile.TileContext(nc) as tc:
        with (
            tc.tile_pool(name="persist", bufs=1) as pp,
            tc.tile_pool(name="gather", bufs=3) as gp,
            tc.tile_pool(name="scratch", bufs=2) as sp,
            tc.tile_pool(name="small", bufs=2) as mp,
        ):
            xh, rln = {}, {}
            xcat = {}  # h -> [128, 256]: [l*rlnT_l | ab*rlnT_ab]
            for h in (0, 1):
                xc = pp.tile([P, D2], F32, tag=f"xcat_{h}")
                for mod, src, off in (("l", l_in, 0), ("ab", ab_in, D)):
                    xt = pp.tile([P, D], F32, tag=f"x_{mod}_{h}")
                    nc.sync.dma_start(out=xt[:], in_=src[h * P:(h + 1) * P, :])
                    ss = pp.tile([P, 1], F32, tag=f"ss_{mod}_{h}")
                    scr = sp.tile([P, D], F32, tag="normscr")
                    nc.scalar.activation(
                        out=scr[:], in_=xt[:],
                        func=mybir.ActivationFunctionType.Square,
                        accum_out=ss[:],
                    )
                    nrm = pp.tile([P, 1], F32, tag=f"nrm_{mod}_{h}")
                    nc.scalar.sqrt(out=nrm[:], in_=ss[:])
                    nc.vector.tensor_scalar_add(out=nrm[:], in0=nrm[:], scalar1=EPS)
                    rt = pp.tile([P, 1], F32, tag=f"rln_{mod}_{h}")
                    nc.vector.reciprocal(out=rt[:], in_=nrm[:])
                    rtT = pp.tile([P, 1], F32, tag=f"rlnT_{mod}_{h}")
                    nc.vector.tensor_scalar_mul(out=rtT[:], in0=rt[:], scalar1=1.0 / T)
                    nc.vector.tensor_scalar_mul(
                        out=xc[:, off:off + D], in0=xt[:], scalar1=rtT[:]
                    )
                    xh[(mod, h)] = xt
                    rln[(mod, h)] = rt
                xcat[h] = xc

            # scores: one gather per column fetches both banks' row; dots for
            # both modalities fall out of one multiply + segmented reduce.
            for h in (0, 1):
                e_l_t = pp.tile([P, C], F32, tag=f"E_l_{h}")
                e_ab_t = pp.tile([P, C], F32, tag=f"E_ab_{h}")
                for (c0, cg) in chunks:
                    it = mp.tile([P, CG], I32, tag="idx")
                    nc.sync.dma_start(
                        out=it[:, :cg],
                        in_=idx_in[h * P:(h + 1) * P, c0:c0 + cg],
                    )
                    g = gp.tile([P, CG * D2], F32, tag="g")
                    # HW indirect DMA honors one offset per partition: one
                    # 1024B gather per score column into slices of one tile.
                    for j in range(cg):
                        nc.gpsimd.indirect_dma_start(
                            out=g[:, j * D2:(j + 1) * D2],
                            out_offset=None,
                            in_=mem_cat[:],
                            in_offset=bass.IndirectOffsetOnAxis(
                                ap=it[:, j:j + 1], axis=0,
                            ),
                        )
                    g3 = g[:, :cg * D2].rearrange("p (c d) -> p c d", d=D2)
                    m = sp.tile([P, CG * D2], F32, tag="m")
                    nc.vector.tensor_tensor(
                        out=m[:, :cg * D2],
                        in0=g3,
                        in1=bcast_chunk(xcat[h][:], cg),
                        op=mybir.AluOpType.mult,
                    )
                    s = mp.tile([P, CG * 2], F32, tag="s")
                    nc.vector.tensor_reduce(
                        out=s[:, :cg * 2],
                        in_=m[:, :cg * D2].rearrange(
                            "p (c two d) -> p c two d", two=2, d=D
                        ),
                        axis=mybir.AxisListType.X,
                        op=mybir.AluOpType.add,
                    )
                    s2 = s[:, :cg * 2].rearrange("p (c two) -> p c two", two=2)
                    nc.scalar.activation(
                        out=e_l_t[:, c0:c0 + cg],
                        in_=s2[:, :, 0],
                        func=mybir.ActivationFunctionType.Exp,
                    )
                    nc.scalar.activation(
                        out=e_ab_t[:, c0:c0 + cg],
                        in_=s2[:, :, 1],
                        func=mybir.ActivationFunctionType.Exp,
                    )
                nc.sync.dma_start(out=out_l[h * P:(h + 1) * P, :], in_=e_l_t[:])
                nc.sync.dma_start(out=out_ab[h * P:(h + 1) * P, :], in_=e_ab_t[:])

            # momentum updates: every core computes all 256 rows
            for h in (0, 1):
                yt = mp.tile([P, 1], I32, tag="y")
                nc.sync.dma_start(out=yt[:], in_=y_in[h * P:(h + 1) * P, :])
                gy = sp.tile([P, D2], F32, tag="gy")
                nc.gpsimd.indirect_dma_start(
                    out=gy[:], out_offset=None, in_=mem_cat[:],
                    in_offset=bass.IndirectOffsetOnAxis(ap=yt[:, :1], axis=0),
                )
                # cat layout: [:, :D] = memory_ab row, [:, D:] = memory_l row
                for mod, off, outd in (("l", D, upd_l), ("ab", 0, upd_ab)):
                    xn = sp.tile([P, D], F32, tag="xn")
                    nc.vector.tensor_scalar_mul(
                        out=xn[:], in0=xh[(mod, h)][:], scalar1=rln[(mod, h)][:]
                    )
                    pos = sp.tile([P, D], F32, tag="pos")
                    nc.vector.tensor_tensor(
                        out=pos[:], in0=gy[:, off:off + D], in1=xn[:],
                        op=mybir.AluOpType.add,
                    )
                    nc.vector.tensor_scalar_mul(out=pos[:], in0=pos[:], scalar1=0.5)
                    scr2 = sp.tile([P, D], F32, tag="scr2")
                    ssu = mp.tile([P, 1], F32, tag="ssu")
                    nc.scalar.activation(
                        out=scr2[:], in_=pos[:],
                        func=mybir.ActivationFunctionType.Square,
                        accum_out=ssu[:],
                    )
                    nrmu = mp.tile([P, 1], F32, tag="nrmu")
                    nc.scalar.sqrt(out=nrmu[:], in_=ssu[:])
                    rnu = mp.tile([P, 1], F32, tag="rnu")
                    nc.vector.reciprocal(out=rnu[:], in_=nrmu[:])
                    nc.vector.tensor_scalar_mul(out=pos[:], in0=pos[:], scalar1=rnu[:])
                    nc.sync.dma_start(out=outd[h * P:(h + 1) * P, :], in_=pos[:])

    if split:
        _split_multiwaits(nc)
    return nc


_cache = {}


def kernel(l, ab, memory_l, memory_ab, y, idx, _trace=False):
    from concourse import bass_utils

    if "nc" not in _cache:
        _cache["nc"] = _build_nc()
    nc = _cache["nc"]

    l = np.ascontiguousarray(np.asarray(l, dtype=np.float32))
    ab = np.ascontiguousarray(np.asarray(ab, dtype=np.float32))
    memory_l = np.asarray(memory_l, dtype=np.float32)
    memory_ab = np.asarray(memory_ab, dtype=np.float32)
    y = np.asarray(y, dtype=np.int32)
    idx = np.asarray(idx, dtype=np.int32)

    mem_cat = np.concatenate([memory_ab, memory_l], axis=1)

    in_maps = []
    for i in range(N_CORES):
        k0 = i * STRIDE
        in_maps.append({
            "mem_cat": mem_cat,
            "l": l,
            "ab": ab,
            "idx_slice": np.ascontiguousarray(idx[:, k0:k0 + C]),
            "y": y[:, None],
        })

    res = bass_utils.run_bass_kernel_spmd(
        nc, in_maps, core_ids=list(range(N_CORES)), trace=_trace
    )
    results = res.results
    _cache["last_exec_time_ns"] = res.exec_time_ns

    def stitch(name):
        full = np.empty((B, KP1), np.float32)
        for i in range(N_CORES):
            full[:, i * STRIDE:i * STRIDE + STRIDE] = results[i][name][:, :STRIDE]
        full[:, KP1 - 1] = results[N_CORES - 1][name][:, STRIDE]
        return full

    e_l = stitch("out_l")
    e_ab = stitch("out_ab")
    z_l = np.float32(e_l.mean(dtype=np.float64) * NB)
    z_ab = np.float32(e_ab.mean(dtype=np.float64) * NB)
    out_l = (e_l / z_l)[..., None].astype(np.float32)
    out_ab = (e_ab / z_ab)[..., None].astype(np.float32)

    new_l = memory_l.copy()
    new_l[y] = results[0]["upd_l"]
    new_ab = memory_ab.copy()
    new_ab[y] = results[0]["upd_ab"]
    return out_l, out_ab, new_l, new_ab
